# revision 29
# baseline (speedup 1.0000x reference)
"""Multi-head attention kernel for Trainium2 (Bass/Tile), 8-core SPMD.

Problem: Q,K,V [B=2, H=16, S=4096, D=64] fp32 -> softmax(Q K^T / sqrt(D)) V.
Sharding: batch*heads (32) split 4-per-core across 8 NeuronCores; each core
computes its heads independently (no collectives).

The baseline bottleneck was the ACT (scalar) engine: softmax needs S*S exps
per head and ACT runs 1 elem/cycle/lane @1.2GHz (~95% busy). This version
splits the exp work across TWO engines:

  scoresT[k,q] = K[k,:] . Q'[q,:]      (PE fp16, Q' pre-scaled by A/sqrt(D),
                                        A = 1024/ln2, so PSUM holds A*score;
                                        row-tiled pairs: two k-chunks run
                                        concurrently in the 128x128 array)
  pT = C*exp(score) in fp16 (C = 1/16), split per score tile:
    - ACT part:  activation(Exp, scale=ln2/1024, bias=-4*ln2) -> fp16
    - DVE part:  tensor_scalar(max(x,-B), +B) -> int16, round-to-nearest;
                 the int16 bits ARE the fp16 encoding of C*exp(score)
                 (Schraudolph bit-trick: fp16 value of bits v is
                 2^((v-15360)/1024)*(1+eps), B = 15360 - 4*1024 = 11264;
                 |eps| <= 2^-11 + PWL error ~ 4e-4. Saturation/NaN are
                 structurally impossible: max p bits 20864 + B < 32767.)
  accT[d,q] += Vaug[k,d] . pT[k,q]     (PE fp16; Vaug row 64 == ones so acc
                                        row 64 accumulates the denominator)
  oT[d,q] = accT[d,q] * recip(accT[64,q])
                                       (DVE reciprocal_approx_fast on an
                                        SBUF-staged copy (PSUM source of the
                                        custom op misbehaves), GpSimd
                                        partition broadcast, DVE mul)
Host side only re-lays-out data (transposes, fp16 casts, V augmentation).
"""

import numpy as np
from contextlib import ExitStack
from collections import deque

import concourse.bacc as bacc
import concourse.bass as bass
import concourse.tile as tile
import concourse.mybir as mybir
from concourse.bass_utils import run_bass_kernel_spmd

F32 = mybir.dt.float32
F16 = mybir.dt.float16
I16 = mybir.dt.int16
EXP = mybir.ActivationFunctionType.Exp
ALU = mybir.AluOpType

B, H, S, D = 2, 16, 4096, 64
N_CORES = 8
HPC = (B * H) // N_CORES  # heads per core

QTILE = 512            # q columns processed per inner iteration
CHUNK = 128            # k rows per matmul (PE partition dim)
NCH = S // CHUNK       # 32 k-chunks per qtile

A_SCALE = 1024.0 / np.log(2.0)       # exponent-arg scale (fp16 has 10 mantissa bits)
B_CONST = 11264.0                    # 15360 - 4*1024 -> C = 2^-4
# ACT-path bias: exp out = e^s/16 times 1.0407 -- the mean of the DVE path's
# Schraudolph ratio (1+f)/2^f over uniform f, so both paths carry the same
# mean factor and it cancels in the softmax normalization
EXP_BIAS = float(-4.0 * np.log(2.0) + np.log(1.040684))

# per-qtile score tiles: 10 tiles of 3 chunks + 1 of 2
TILE_SIZES = [3] * 10 + [2]
# ACT/DVE split point within a tile's flattened free dim (multiple of 16);
# f ~ 0.57 balances ACT (overhead ~257cyc @1.2GHz) vs DVE (~120cyc @0.96GHz
# at 1x PSUM-source rate, plus the epilogue ops)
ACT_M = {3: 880, 2: 592}
MM2_LAG = 2  # tiles between exp issue and the consuming MM2 issue


def build_nc(hpc: int = HPC, s: int = S, qtile: int = QTILE):
    n_qtiles = s // qtile
    nch = s // CHUNK

    nc = bacc.Bacc("TRN2", target_bir_lowering=False, debug=False)
    qt_d = nc.dram_tensor("qt", [hpc, D, s], F16, kind="ExternalInput").ap()
    kt_d = nc.dram_tensor("kt", [hpc, D, s], F16, kind="ExternalInput").ap()
    va_d = nc.dram_tensor("va", [hpc, s, D + 1], F16, kind="ExternalInput").ap()
    o_d = nc.dram_tensor("o", [hpc, D, s], F32, kind="ExternalOutput").ap()

    with tile.TileContext(nc) as tc, ExitStack() as ctx:
        qk_pool = ctx.enter_context(tc.tile_pool(name="qk", bufs=2))
        v_pool = ctx.enter_context(tc.tile_pool(name="v", bufs=2))
        p16_pool = ctx.enter_context(tc.tile_pool(name="p16", bufs=2))
        ot_pool = ctx.enter_context(tc.tile_pool(name="ot", bufs=2))
        small_pool = ctx.enter_context(tc.tile_pool(name="small", bufs=6))
        const_pool = ctx.enter_context(tc.tile_pool(name="const", bufs=1))
        sc_psum = ctx.enter_context(tc.tile_pool(name="sc", bufs=2, space="PSUM"))
        oa_psum = ctx.enter_context(tc.tile_pool(name="oa", bufs=2, space="PSUM"))

        # exp bias as a per-partition const AP (activation requires an AP bias)
        bias_sb = const_pool.tile([128, 1], F32)
        nc.vector.memset(bias_sb[:], EXP_BIAS)
        # prewarm the ACT exp table set while the first DMAs are in flight
        warm = const_pool.tile([1, 1], F32)
        nc.vector.memset(warm[:], 0.0)
        warm2 = const_pool.tile([1, 1], F16)
        nc.scalar.activation(warm2[:], warm[:], EXP, scale=1.0, bias=bias_sb[0:1, :])

        # deferred work: MM2 chunks lag exp issue by MM2_LAG tiles so the PE
        # (in-order queue) never parks on a matmul whose exp input isn't done
        gtile = 0
        deferred = deque()  # (ready_gtile, kind, payload)

        def issue(kind, payload):
            if kind == "mm2":
                h_, acc_, va_, p16v_, c_ = payload
                nc.tensor.matmul(
                    acc_[:], va_[:, c_, :], p16v_[:, c_, :],
                    start=(c_ == 0), stop=(c_ == nch - 1),
                )
            else:  # epilogue
                h_, qt_, acc_, ot_ = payload
                qs_ = slice(qt_ * qtile, (qt_ + 1) * qtile)
                den_s = small_pool.tile([1, qtile], F32, tag="den")
                nc.vector.tensor_copy(den_s[:], acc_[D : D + 1, :])
                r_sb = small_pool.tile([1, qtile], F32, tag="r")
                # NB: reciprocal_approx_fast misbehaves on a PSUM source
                # (BITWISE_NOT seed path) -- feed it from SBUF only
                nc.vector.reciprocal_approx_fast(r_sb[:], den_s[:])
                bc = small_pool.tile([D, qtile], F32, tag="bc")
                nc.gpsimd.partition_broadcast(bc[:], r_sb[:])
                nc.vector.tensor_mul(ot_[:, qs_], acc_[0:D, :], bc[:])
                # SWDGE path so input loads on the HWDGE ring aren't queued
                # behind stores
                nc.gpsimd.dma_start(o_d[h_][:, qs_], ot_[:, qs_])

        def flush(lag):
            while deferred and deferred[0][0] <= gtile - lag:
                _, kind, payload = deferred.popleft()
                issue(kind, payload)

        for h in range(hpc):
            # K^T and Q^T [D, s] duplicated into both partition halves so two
            # k-chunks can run concurrently via PE row tiling.
            qt_sb = qk_pool.tile([128, s], F16, tag="qt")
            kt_sb = qk_pool.tile([128, s], F16, tag="kt")
            va_sb = v_pool.tile([128, nch, D + 1], F16)
            va_r = va_d[h].rearrange("(c p) e -> p c e", p=128)
            # tiered loads: small leading slices of everything first, then
            # interleaved k/V pieces (consumed in chunk order), q tails last
            kcut = min(8 * CHUNK, s)
            ncut = kcut // CHUNK
            nc.sync.dma_start(kt_sb[0:D, 0:kcut], kt_d[h][:, 0:kcut])
            nc.sync.dma_start(qt_sb[0:D, 0:qtile], qt_d[h][:, 0:qtile])
            nc.sync.dma_start(kt_sb[D : 2 * D, 0:kcut], kt_d[h][:, 0:kcut])
            nc.sync.dma_start(qt_sb[D : 2 * D, 0:qtile], qt_d[h][:, 0:qtile])
            nc.sync.dma_start(va_sb[:, 0:ncut, :], va_r[:, 0:ncut, :])
            cuts = [kcut]
            while cuts[-1] < s:
                cuts.append(min(cuts[-1] + 12 * CHUNK, s))
            for c0_, c1_ in zip(cuts, cuts[1:]):
                n0_, n1_ = c0_ // CHUNK, c1_ // CHUNK
                nc.sync.dma_start(kt_sb[0:D, c0_:c1_], kt_d[h][:, c0_:c1_])
                nc.sync.dma_start(kt_sb[D : 2 * D, c0_:c1_], kt_d[h][:, c0_:c1_])
                nc.sync.dma_start(va_sb[:, n0_:n1_, :], va_r[:, n0_:n1_, :])
            if qtile < s:
                nc.sync.dma_start(qt_sb[0:D, qtile:s], qt_d[h][:, qtile:s])
                nc.sync.dma_start(qt_sb[D : 2 * D, qtile:s], qt_d[h][:, qtile:s])
            o_t = ot_pool.tile([D, s], F32)

            for qt in range(n_qtiles):
                qs = slice(qt * qtile, (qt + 1) * qtile)
                acc = oa_psum.tile([D + 1, qtile], F32)
                p16t = p16_pool.tile([128, nch * qtile], F16, tag="p16")
                p16i = p16t[:].bitcast(I16)
                p16v = p16t[:].rearrange("p (c q) -> p c q", q=qtile)
                c0 = 0
                chunk_issued = 0
                for t, gs in enumerate(TILE_SIZES):
                    sc = sc_psum.tile([128, 3 * qtile], F32, tag="sc")
                    for j in range(gs):
                        c = c0 + j
                        # alternate partition halves by global chunk index so
                        # consecutive chunks run concurrently via PE row tiling
                        half = slice(0, D) if c % 2 == 0 else slice(D, 2 * D)
                        nc.tensor.matmul(
                            sc[:, j * qtile : (j + 1) * qtile],
                            kt_sb[half, c * CHUNK : (c + 1) * CHUNK],
                            qt_sb[half, qs],
                            start=True, stop=True,
                        )
                    nw = gs * qtile
                    m = ACT_M[gs]
                    base = c0 * qtile
                    # exp split: ACT does [0:m) natively to fp16, DVE does
                    # [m:nw) via the int16 Schraudolph trick (same value space)
                    nc.scalar.activation(
                        p16t[:, base : base + m], sc[:, 0:m], EXP,
                        scale=float(np.log(2.0) / 1024.0), bias=bias_sb[:],
                    )
                    nc.vector.tensor_scalar(
                        p16i[:, base + m : base + nw], sc[:, m:nw],
                        -B_CONST, B_CONST, ALU.max, ALU.add,
                    )
                    c0 += gs
                    gtile += 1
                    while chunk_issued < c0:
                        deferred.append(
                            (gtile, "mm2", (h, acc, va_sb, p16v, chunk_issued))
                        )
                        chunk_issued += 1
                    if t == len(TILE_SIZES) - 1:
                        deferred.append((gtile, "epi", (h, qt, acc, o_t)))
                    flush(MM2_LAG)
        while deferred:
            _, kind, payload = deferred.popleft()
            issue(kind, payload)

    nc.compile()
    return nc


_NC_CACHE = {}


def _get_nc(hpc=HPC, s=S, qtile=QTILE):
    key = (hpc, s, qtile)
    if key not in _NC_CACHE:
        _NC_CACHE[key] = build_nc(hpc, s, qtile)
    return _NC_CACHE[key]


def prep_inputs(Q, K, V):
    """Host-side re-layout: per-core input maps."""
    bh = B * H
    # Q pre-scaled by A/sqrt(D) so PSUM scores arrive in exponent-arg space
    q2 = np.ascontiguousarray(
        np.asarray(Q, dtype=np.float32).reshape(bh, S, D).transpose(0, 2, 1)
        * np.float32(A_SCALE / np.sqrt(D))
    ).astype(np.float16)
    k2 = np.ascontiguousarray(
        np.asarray(K, dtype=np.float32).reshape(bh, S, D).transpose(0, 2, 1)
    ).astype(np.float16)
    v = np.asarray(V, dtype=np.float32).reshape(bh, S, D).astype(np.float16)
    va = np.concatenate([v, np.ones((bh, S, 1), dtype=np.float16)], axis=-1)
    in_maps = []
    for c in range(N_CORES):
        sl = slice(c * HPC, (c + 1) * HPC)
        in_maps.append({
            "qt": np.ascontiguousarray(q2[sl]),
            "kt": np.ascontiguousarray(k2[sl]),
            "va": np.ascontiguousarray(va[sl]),
        })
    return in_maps


def run(Q, K, V, trace=False, **kwargs):
    nc = _get_nc()
    in_maps = prep_inputs(Q, K, V)
    res = run_bass_kernel_spmd(
        nc, in_maps, core_ids=list(range(N_CORES)), trace=trace, **kwargs
    )
    # o is [hpc, D, s] per core -> transpose back to [hpc, s, D]
    outs = [
        np.ascontiguousarray(res.results[c]["o"].transpose(0, 2, 1))
        for c in range(N_CORES)
    ]
    full = np.concatenate(outs, axis=0).reshape(B, H, S, D)
    return full, res


def kernel(Q, K, V):
    # retry on transient device/runtime errors (e.g. a wedged NeuronCore
    # left over from a previous run recovers on re-execution)
    import time
    last = None
    for attempt in range(3):
        try:
            out, _ = run(Q, K, V)
            return out
        except Exception as e:  # noqa: BLE001
            last = e
            time.sleep(5)
    raise last


# revision 32
# speedup vs baseline: 1.0097x; 1.0097x over previous
"""Multi-head attention kernel for Trainium2 (Bass/Tile), 8-core SPMD.

Problem: Q,K,V [B=2, H=16, S=4096, D=64] fp32 -> softmax(Q K^T / sqrt(D)) V.
Sharding: batch*heads (32) split 4-per-core across 8 NeuronCores; each core
computes its heads independently (no collectives).

The baseline bottleneck was the ACT (scalar) engine: softmax needs S*S exps
per head and ACT runs 1 elem/cycle/lane @1.2GHz (~95% busy). This version
splits the exp work across TWO engines:

  scoresT[k,q] = K[k,:] . Q'[q,:]      (PE fp16, Q' pre-scaled by A/sqrt(D),
                                        A = 1024/ln2, so PSUM holds A*score;
                                        row-tiled pairs: two k-chunks run
                                        concurrently in the 128x128 array)
  pT = C*exp(score) in fp16 (C = 1/16), split per score tile:
    - ACT part:  activation(Exp, scale=ln2/1024, bias=-4*ln2) -> fp16
    - DVE part:  tensor_scalar(max(x,-B), +B) -> int16, round-to-nearest;
                 the int16 bits ARE the fp16 encoding of C*exp(score)
                 (Schraudolph bit-trick: fp16 value of bits v is
                 2^((v-15360)/1024)*(1+eps), B = 15360 - 4*1024 = 11264;
                 |eps| <= 2^-11 + PWL error ~ 4e-4. Saturation/NaN are
                 structurally impossible: max p bits 20864 + B < 32767.)
  accT[d,q] += Vaug[k,d] . pT[k,q]     (PE fp16; Vaug row 64 == ones so acc
                                        row 64 accumulates the denominator)
  oT[d,q] = accT[d,q] * recip(accT[64,q])
                                       (DVE reciprocal_approx_fast on an
                                        SBUF-staged copy (PSUM source of the
                                        custom op misbehaves), GpSimd
                                        partition broadcast, DVE mul)
Host side only re-lays-out data (transposes, fp16 casts, V augmentation).
"""

import numpy as np
from contextlib import ExitStack
from collections import deque

import concourse.bacc as bacc
import concourse.bass as bass
import concourse.tile as tile
import concourse.mybir as mybir
from concourse.bass_utils import run_bass_kernel_spmd

F32 = mybir.dt.float32
F16 = mybir.dt.float16
I16 = mybir.dt.int16
EXP = mybir.ActivationFunctionType.Exp
ALU = mybir.AluOpType

B, H, S, D = 2, 16, 4096, 64
N_CORES = 8
HPC = (B * H) // N_CORES  # heads per core

QTILE = 512            # q columns processed per inner iteration
CHUNK = 128            # k rows per matmul (PE partition dim)
NCH = S // CHUNK       # 32 k-chunks per qtile

A_SCALE = 1024.0 / np.log(2.0)       # exponent-arg scale (fp16 has 10 mantissa bits)
B_CONST = 11264.0                    # 15360 - 4*1024 -> C = 2^-4
# ACT-path bias: exp out = e^s/16 times 1.0407 -- the mean of the DVE path's
# Schraudolph ratio (1+f)/2^f over uniform f, so both paths carry the same
# mean factor and it cancels in the softmax normalization
EXP_BIAS = float(-4.0 * np.log(2.0) + np.log(1.040684))

# per-qtile score tiles: 10 tiles of 3 chunks + 1 of 2
TILE_SIZES = [3] * 10 + [2]
# ACT/DVE split point within a tile's flattened free dim (multiple of 16);
# f ~ 0.57 balances ACT (overhead ~257cyc @1.2GHz) vs DVE (~120cyc @0.96GHz
# at 1x PSUM-source rate, plus the epilogue ops)
ACT_M = {3: 880, 2: 592}
MM2_LAG = 2  # tiles between exp issue and the consuming MM2 issue


def build_nc(hpc: int = HPC, s: int = S, qtile: int = QTILE):
    n_qtiles = s // qtile
    nch = s // CHUNK

    nc = bacc.Bacc("TRN2", target_bir_lowering=False, debug=False)
    qt_d = nc.dram_tensor("qt", [hpc, D, s], F16, kind="ExternalInput").ap()
    kt_d = nc.dram_tensor("kt", [hpc, D, s], F16, kind="ExternalInput").ap()
    va_d = nc.dram_tensor("va", [hpc, s, D + 1], F16, kind="ExternalInput").ap()
    o_d = nc.dram_tensor("o", [hpc, D, s], F32, kind="ExternalOutput").ap()

    with tile.TileContext(nc) as tc, ExitStack() as ctx:
        qk_pool = ctx.enter_context(tc.tile_pool(name="qk", bufs=2))
        v_pool = ctx.enter_context(tc.tile_pool(name="v", bufs=2))
        p16_pool = ctx.enter_context(tc.tile_pool(name="p16", bufs=2))
        ot_pool = ctx.enter_context(tc.tile_pool(name="ot", bufs=2))
        small_pool = ctx.enter_context(tc.tile_pool(name="small", bufs=6))
        const_pool = ctx.enter_context(tc.tile_pool(name="const", bufs=1))
        sc_psum = ctx.enter_context(tc.tile_pool(name="sc", bufs=2, space="PSUM"))
        oa_psum = ctx.enter_context(tc.tile_pool(name="oa", bufs=2, space="PSUM"))

        # exp bias as a per-partition const AP (activation requires an AP bias)
        bias_sb = const_pool.tile([128, 1], F32)
        nc.vector.memset(bias_sb[:], EXP_BIAS)
        # prewarm the ACT exp table set while the first DMAs are in flight
        warm = const_pool.tile([1, 1], F32)
        nc.vector.memset(warm[:], 0.0)
        warm2 = const_pool.tile([1, 1], F16)
        nc.scalar.activation(warm2[:], warm[:], EXP, scale=1.0, bias=bias_sb[0:1, :])

        # deferred work: MM2 chunks lag exp issue by MM2_LAG tiles so the PE
        # (in-order queue) never parks on a matmul whose exp input isn't done
        gtile = 0
        deferred = deque()  # (ready_gtile, kind, payload)

        def issue(kind, payload):
            if kind == "mm2":
                h_, acc_, va_, p16v_, c_ = payload
                nc.tensor.matmul(
                    acc_[:], va_[:, c_, :], p16v_[:, c_, :],
                    start=(c_ == 0), stop=(c_ == nch - 1),
                )
            else:  # epilogue
                h_, qt_, acc_, ot_ = payload
                qs_ = slice(qt_ * qtile, (qt_ + 1) * qtile)
                den_s = small_pool.tile([1, qtile], F32, tag="den")
                nc.vector.tensor_copy(den_s[:], acc_[D : D + 1, :])
                r_sb = small_pool.tile([1, qtile], F32, tag="r")
                # NB: reciprocal_approx_fast misbehaves on a PSUM source
                # (BITWISE_NOT seed path) -- feed it from SBUF only
                nc.vector.reciprocal_approx_fast(r_sb[:], den_s[:])
                bc = small_pool.tile([D, qtile], F32, tag="bc")
                nc.gpsimd.partition_broadcast(bc[:], r_sb[:])
                nc.vector.tensor_mul(ot_[:, qs_], acc_[0:D, :], bc[:])
                # SWDGE path so input loads on the HWDGE ring aren't queued
                # behind stores
                nc.gpsimd.dma_start(o_d[h_][:, qs_], ot_[:, qs_])

        def flush(lag):
            while deferred and deferred[0][0] <= gtile - lag:
                _, kind, payload = deferred.popleft()
                issue(kind, payload)

        for h in range(hpc):
            # K^T and Q^T [D, s] duplicated into both partition halves so two
            # k-chunks can run concurrently via PE row tiling.
            qt_sb = qk_pool.tile([128, s], F16, tag="qt")
            kt_sb = qk_pool.tile([128, s], F16, tag="kt")
            va_sb = v_pool.tile([128, nch, D + 1], F16)
            va_r = va_d[h].rearrange("(c p) e -> p c e", p=128)
            # tiered loads: small leading slices of everything first, then
            # interleaved k/V pieces (consumed in chunk order), q tails last
            kcut = min(8 * CHUNK, s)
            ncut = kcut // CHUNK
            nc.sync.dma_start(kt_sb[0:D, 0:kcut], kt_d[h][:, 0:kcut])
            nc.sync.dma_start(qt_sb[0:D, 0:qtile], qt_d[h][:, 0:qtile])
            nc.sync.dma_start(kt_sb[D : 2 * D, 0:kcut], kt_d[h][:, 0:kcut])
            nc.sync.dma_start(qt_sb[D : 2 * D, 0:qtile], qt_d[h][:, 0:qtile])
            nc.sync.dma_start(va_sb[:, 0:ncut, :], va_r[:, 0:ncut, :])
            cuts = [kcut]
            while cuts[-1] < s:
                cuts.append(min(cuts[-1] + 12 * CHUNK, s))
            for c0_, c1_ in zip(cuts, cuts[1:]):
                n0_, n1_ = c0_ // CHUNK, c1_ // CHUNK
                nc.sync.dma_start(kt_sb[0:D, c0_:c1_], kt_d[h][:, c0_:c1_])
                nc.sync.dma_start(kt_sb[D : 2 * D, c0_:c1_], kt_d[h][:, c0_:c1_])
                nc.sync.dma_start(va_sb[:, n0_:n1_, :], va_r[:, n0_:n1_, :])
            if qtile < s:
                nc.sync.dma_start(qt_sb[0:D, qtile:s], qt_d[h][:, qtile:s])
                nc.sync.dma_start(qt_sb[D : 2 * D, qtile:s], qt_d[h][:, qtile:s])
            o_t = ot_pool.tile([D, s], F32)

            for qt in range(n_qtiles):
                qs = slice(qt * qtile, (qt + 1) * qtile)
                acc = oa_psum.tile([D + 1, qtile], F32)
                p16t = p16_pool.tile([128, nch * qtile], F16, tag="p16")
                p16i = p16t[:].bitcast(I16)
                p16v = p16t[:].rearrange("p (c q) -> p c q", q=qtile)
                c0 = 0
                chunk_issued = 0
                for t, gs in enumerate(TILE_SIZES):
                    sc = sc_psum.tile([128, 3 * qtile], F32, tag="sc")
                    for j in range(gs):
                        c = c0 + j
                        # alternate partition halves by global chunk index so
                        # consecutive chunks run concurrently via PE row tiling
                        half = slice(0, D) if c % 2 == 0 else slice(D, 2 * D)
                        nc.tensor.matmul(
                            sc[:, j * qtile : (j + 1) * qtile],
                            kt_sb[half, c * CHUNK : (c + 1) * CHUNK],
                            qt_sb[half, qs],
                            start=True, stop=True,
                        )
                    nw = gs * qtile
                    m = ACT_M[gs]
                    base = c0 * qtile
                    # exp split: ACT does [0:m) natively to fp16, DVE does
                    # [m:nw) via the int16 Schraudolph trick (same value space)
                    nc.scalar.activation(
                        p16t[:, base : base + m], sc[:, 0:m], EXP,
                        scale=float(np.log(2.0) / 1024.0), bias=bias_sb[:],
                    )
                    nc.vector.tensor_scalar(
                        p16i[:, base + m : base + nw], sc[:, m:nw],
                        -B_CONST, B_CONST, ALU.max, ALU.add,
                    )
                    c0 += gs
                    gtile += 1
                    while chunk_issued < c0:
                        deferred.append(
                            (gtile, "mm2", (h, acc, va_sb, p16v, chunk_issued))
                        )
                        chunk_issued += 1
                    if t == len(TILE_SIZES) - 1:
                        deferred.append((gtile, "epi", (h, qt, acc, o_t)))
                    flush(MM2_LAG)
        while deferred:
            _, kind, payload = deferred.popleft()
            issue(kind, payload)

    nc.compile()
    return nc


_NC_CACHE = {}


def _get_nc(hpc=HPC, s=S, qtile=QTILE):
    key = (hpc, s, qtile)
    if key not in _NC_CACHE:
        _NC_CACHE[key] = build_nc(hpc, s, qtile)
    return _NC_CACHE[key]


def prep_inputs(Q, K, V):
    """Host-side re-layout: per-core input maps."""
    bh = B * H
    # Q pre-scaled by A/sqrt(D) so PSUM scores arrive in exponent-arg space
    q2 = np.ascontiguousarray(
        np.asarray(Q, dtype=np.float32).reshape(bh, S, D).transpose(0, 2, 1)
        * np.float32(A_SCALE / np.sqrt(D))
    ).astype(np.float16)
    k2 = np.ascontiguousarray(
        np.asarray(K, dtype=np.float32).reshape(bh, S, D).transpose(0, 2, 1)
    ).astype(np.float16)
    v = np.asarray(V, dtype=np.float32).reshape(bh, S, D).astype(np.float16)
    va = np.concatenate([v, np.ones((bh, S, 1), dtype=np.float16)], axis=-1)
    in_maps = []
    for c in range(N_CORES):
        sl = slice(c * HPC, (c + 1) * HPC)
        in_maps.append({
            "qt": np.ascontiguousarray(q2[sl]),
            "kt": np.ascontiguousarray(k2[sl]),
            "va": np.ascontiguousarray(va[sl]),
        })
    return in_maps


def run(Q, K, V, trace=False, **kwargs):
    nc = _get_nc()
    in_maps = prep_inputs(Q, K, V)
    res = run_bass_kernel_spmd(
        nc, in_maps, core_ids=list(range(N_CORES)), trace=trace, **kwargs
    )
    # o is [hpc, D, s] per core -> transpose back to [hpc, s, D]
    outs = [
        np.ascontiguousarray(res.results[c]["o"].transpose(0, 2, 1))
        for c in range(N_CORES)
    ]
    full = np.concatenate(outs, axis=0).reshape(B, H, S, D)
    return full, res


def kernel(Q, K, V):
    # retry on transient device/runtime errors (e.g. a wedged NeuronCore
    # left over from a previous run recovers on re-execution)
    import time
    last = None
    for attempt in range(3):
        try:
            out, _ = run(Q, K, V)
            return out
        except Exception as e:  # noqa: BLE001
            last = e
            time.sleep(5)
    raise last
